# revision 6
# baseline (speedup 1.0000x reference)
"""ColBERT negative-CE loss on 8 Trainium2 NeuronCores (Bass/Tile).

Problem (hardcoded shapes): B=64, N=32 query tokens, S=1024 doc tokens, D=128.
  pos/neg paired MaxSim + in-batch (b x c) MaxSim cross-entropy, T=0.02.

Strategy (v2):
  * Shard the in-batch score matrix by DOC COLUMNS: core r computes
    scores[:, r*8:(r+1)*8] (all 64 query rows vs its 8 docs) plus the paired
    neg scores for its own 8 batch rows. pos_scores = diagonal of scores.
  * All matmuls in bf16 (inputs quantized host-side; ~1e-3 final rel err).
  * Each (m-tile, doc) tile = 1024 raw scores per lane in 2 PSUM banks.
    Two consumer pipelines, balanced across engines:
      kappa: ScalarE copies bank B -> SBUF; a CUSTOM DVE op (registered at
        runtime into concourse.dve_ops) computes max(bankA, copyB) and
        max-reduces it in ONE 2-port DVE pass (~0.73us/tile on DVE only).
      beta: host pre-computes half-doc sum/diff; PE computes P=q@hsum,
        Q=q@hdif; ScalarE abs(Q)->bf16 W; PE accumulates I@W onto P
        (max(a,b) = (a+b)/2 + |a-b|/2); DVE reduces two merged banks in one
        [128,2,512] pass (~0.63us/tile DVE, +0.25us PE).
    K_BETA docs per core take the beta path to offload DVE onto PE/ACT.
  * Neg tiles pack 4 batch rows (32 partitions each) into full 128-partition
    banks, handled by the kappa path (2 tile-units for 8 rows).
  * Final sum over the 32 query tokens per row: ones-block matmul ->
    [4, 130] result, DMA out; O(64x64) softmax/softplus epilogue on host.
"""

import numpy as np

B = 64
N = 32  # query tokens per row
S = 1024  # doc tokens
D = 128
NCORES = 8
LB = B // NCORES  # 8 docs (and batch rows) per core
H = S // 2  # 512, half-doc
MT = (B * N) // 128  # 16 m-tiles of 128 query tokens
TEMP = 0.02
K_BETA = 4  # docs 0..K_BETA-1 per core use the beta (sum/diff) path
NCOLS = MT * LB + 2  # 128 in-batch cols + 2 packed neg cols = 130

_NC_CACHE = {}


def _register_ttmax():
    """Register a custom DVE op: out = max(in0, in1); accum_out = row-max."""
    import concourse.dve_ops as dve_ops
    from concourse.dve_spec import AluOp, Spec, Src0, Src1, lower, maxx
    from concourse.dve_uop import DveOpSpec

    for op in dve_ops.OPS:
        if op.name == "TT_MAX_RED_ANT":
            return op

    def _ref(in0, in1, s0, s1, imm2):
        b = np.maximum(in0, in1).astype(np.float32)
        return b, b.reshape(b.shape[0], -1).max(axis=-1, keepdims=True)

    spec = Spec(body=maxx(Src0, Src1), accum=AluOp.MAX, reference=_ref)
    name = "TT_MAX_RED_ANT"
    row = dve_ops._CUSTOM_DVE_ROW_BASE + len(dve_ops.OPS)
    shas = {}
    for ver in ("v3", "v4"):
        uops = lower(spec, ver=ver)
        shas[ver] = DveOpSpec(name=name, opcode=row, uops=uops, rd1_en=True).sha(ver)
    op = dve_ops.DveOp(name, spec, subdim=False, uops_sha=shas)
    dve_ops.OPS.append(op)
    dve_ops._SUB_OPCODE_FOR_NAME[name] = row
    dve_ops.CUSTOM_DVE_SPECS[name] = spec
    return op


def _build_nc2():
    import concourse.bacc as bacc
    import concourse.mybir as mybir
    import concourse.tile as tile

    F32 = mybir.dt.float32
    F32R = mybir.dt.float32r
    BF16 = mybir.dt.bfloat16
    X = mybir.AxisListType.X
    ABS = mybir.ActivationFunctionType.Abs

    TTM = _register_ttmax()

    nc = bacc.Bacc("TRN2", target_bir_lowering=False, debug=False)

    qT = nc.dram_tensor("qT", [128, B * N], BF16, kind="ExternalInput").ap()
    qLocT = nc.dram_tensor("qLocT", [128, LB * N], BF16, kind="ExternalInput").ap()
    dT = nc.dram_tensor("dT", [128, LB * S], BF16, kind="ExternalInput").ap()
    nT = nc.dram_tensor("nT", [128, LB * S], BF16, kind="ExternalInput").ap()
    idenb = nc.dram_tensor("idenb", [128, 128], BF16, kind="ExternalInput").ap()
    ones4 = nc.dram_tensor("ones4", [128, 4], F32, kind="ExternalInput").ap()
    out = nc.dram_tensor("out", [4, NCOLS], F32, kind="ExternalOutput").ap()

    with tile.TileContext(nc) as tc:
        with (
            tc.tile_pool(name="consts", bufs=1) as consts,
            tc.tile_pool(name="wbuf", bufs=3) as wbuf_p,
            tc.tile_pool(name="vbuf", bufs=3) as vbuf_p,
            tc.tile_pool(name="scrp", bufs=2) as scr_p,
            tc.tile_pool(name="psum", bufs=2, space="PSUM") as psum_p,
        ):
            q_t = consts.tile([128, B * N], BF16, tag="q")
            ql_t = consts.tile([128, LB * N], BF16, tag="ql")
            d_t = consts.tile([128, LB * S], BF16, tag="d")
            n_t = consts.tile([128, LB * S], BF16, tag="n")
            id_t = consts.tile([128, 128], BF16, tag="id")
            ones_t = consts.tile([128, 4], F32, tag="ones")
            mx = consts.tile([128, NCOLS], F32, tag="mx")
            nc.vector.memset(mx[:], 0.0)

            nc.sync.dma_start(q_t[:], qT[:])
            nc.sync.dma_start(id_t[:], idenb[:])
            nc.sync.dma_start(ones_t[:], ones4[:])
            for c in range(LB):
                nc.sync.dma_start(
                    d_t[:, c * S : (c + 1) * S], dT[:, c * S : (c + 1) * S]
                )
            nc.sync.dma_start(ql_t[:], qLocT[:])
            nc.sync.dma_start(n_t[:], nT[:])

            wz = consts.tile([128, 128], BF16, tag="wz")
            nc.vector.memset(wz[:].bitcast(mybir.dt.uint16), 0)
            wps = psum_p.tile([128, 2048], F32, tag="pp", name="warm")
            for _ in range(12):
                nc.tensor.matmul(
                    wps[:, 0:128], wz[:], wz[:], start=True, stop=True,
                    skip_group_check=True,
                )

            def q_ap(m):
                return q_t[:, m * 128 : (m + 1) * 128]

            def beta_pair(c, m):
                pp = psum_p.tile([128, 2048], F32, tag="pp")
                hs = d_t[:, c * S : c * S + H]
                hd = d_t[:, c * S + H : (c + 1) * S]
                nc.tensor.matmul(pp[:, 0:512], q_ap(m), hs, start=True, stop=False)
                nc.tensor.matmul(pp[:, 512:1024], q_ap(m + 1), hs, start=True, stop=False)
                nc.tensor.matmul(pp[:, 1024:1536], q_ap(m), hd, start=True, stop=True)
                nc.tensor.matmul(pp[:, 1536:2048], q_ap(m + 1), hd, start=True, stop=True)
                w = wbuf_p.tile([128, 1024], BF16, tag="w")
                nc.scalar.activation(w[:], pp[:, 1024:2048], ABS)
                nc.tensor.matmul(pp[:, 0:512], id_t[:], w[:, 0:512], start=False, stop=True)
                nc.tensor.matmul(pp[:, 512:1024], id_t[:], w[:, 512:1024], start=False, stop=True)
                col = c * MT + m
                nc.vector.reduce_max(
                    mx[:, col : col + 2],
                    pp[:, 0:1024].rearrange("p (w k) -> p w k", w=2),
                    axis=X,
                )

            def kappa_pair(c, m):
                pp = psum_p.tile([128, 2048], F32, tag="pp")
                da = d_t[:, c * S : c * S + H]
                db = d_t[:, c * S + H : (c + 1) * S]
                nc.tensor.matmul(pp[:, 0:512], q_ap(m), da, start=True, stop=True)
                nc.tensor.matmul(pp[:, 512:1024], q_ap(m + 1), da, start=True, stop=True)
                nc.tensor.matmul(pp[:, 1024:1536], q_ap(m), db, start=True, stop=True)
                nc.tensor.matmul(pp[:, 1536:2048], q_ap(m + 1), db, start=True, stop=True)
                v = vbuf_p.tile([128, 1024], F32, tag="v")
                nc.scalar.copy(v[:], pp[:, 1024:2048])
                col = c * MT + m
                scr = scr_p.tile([128, 512], F32, tag="scr")
                nc.vector._custom_dve(
                    TTM,
                    out=scr[:],
                    accum_out=mx[:, col : col + 1],
                    in0=pp[:, 0:512],
                    in1=v[:, 0:512],
                )
                scr2 = scr_p.tile([128, 512], F32, tag="scr")
                nc.vector._custom_dve(
                    TTM,
                    out=scr2[:],
                    accum_out=mx[:, col + 1 : col + 2],
                    in0=pp[:, 512:1024],
                    in1=v[:, 512:1024],
                )

            def neg_units():
                # 8 local rows; bank layout: [A(rows0-3) | A(rows4-7) |
                # B(rows0-3) | B(rows4-7)], each bank 4x[32,512] stacked.
                pp = psum_p.tile([128, 2048], F32, tag="pp")
                for g in range(2):
                    for j in range(4):
                        b = 4 * g + j
                        lhs = ql_t[:, b * N : (b + 1) * N]
                        na = n_t[:, b * S : b * S + H]
                        nb = n_t[:, b * S + H : (b + 1) * S]
                        nc.tensor.matmul(
                            pp[32 * j : 32 * j + 32, g * 512 : (g + 1) * 512],
                            lhs, na, start=True, stop=True,
                            tile_position=(0, 32 * j),
                        )
                        nc.tensor.matmul(
                            pp[32 * j : 32 * j + 32, 1024 + g * 512 : 1024 + (g + 1) * 512],
                            lhs, nb, start=True, stop=True,
                            tile_position=(0, 32 * j),
                        )
                v = vbuf_p.tile([128, 1024], F32, tag="v")
                nc.scalar.copy(v[:], pp[:, 1024:2048])
                for g in range(2):
                    scr = scr_p.tile([128, 512], F32, tag="scr")
                    nc.vector._custom_dve(
                        TTM,
                        out=scr[:],
                        accum_out=mx[:, MT * LB + g : MT * LB + g + 1],
                        in0=pp[:, g * 512 : (g + 1) * 512],
                        in1=v[:, g * 512 : (g + 1) * 512],
                    )

            # interleave beta and kappa docs to keep ACT/DVE/PE balanced
            doc_order = []
            bdocs = list(range(K_BETA))
            kdocs = list(range(K_BETA, LB))
            while bdocs or kdocs:
                if bdocs:
                    doc_order.append(("b", bdocs.pop(0)))
                if kdocs:
                    doc_order.append(("k", kdocs.pop(0)))
                if kdocs:
                    doc_order.append(("k", kdocs.pop(0)))

            for kind, c in doc_order:
                for m in range(0, MT, 2):
                    if kind == "b":
                        beta_pair(c, m)
                    else:
                        kappa_pair(c, m)
            neg_units()

            # final: sum the 32 tokens of each row j via ones-block matmul
            psf = psum_p.tile([128, 2048], F32, tag="pp")
            nc.tensor.matmul(
                psf[0:4, 0:112], ones_t[:], mx[:, 0:112], start=True, stop=True,
            )
            nc.tensor.matmul(
                psf[0:4, 112:NCOLS], ones_t[:], mx[:, 112:NCOLS],
                start=True, stop=True,
            )
            out_sb = consts.tile([4, NCOLS], F32, tag="outsb")
            nc.scalar.copy(out_sb[:], psf[0:4, 0:NCOLS])
            nc.sync.dma_start(out[:], out_sb[:])

    nc.compile()
    return nc


def get_nc():
    if "nc" not in _NC_CACHE:
        _NC_CACHE["nc"] = _build_nc2()
    return _NC_CACHE["nc"]


def _prep_inputs(q, d, nd):
    """Build the 8 per-core input maps (bf16)."""
    import ml_dtypes

    BF = ml_dtypes.bfloat16
    qtok = np.ascontiguousarray(q.reshape(B * N, D).T).astype(BF)  # (128, 2048)
    idenb = np.eye(128, dtype=np.float32).astype(BF)
    ones4 = (np.arange(128)[:, None] // 32 == np.arange(4)[None, :]).astype(np.float32)

    a, b = d[:, :H, :], d[:, H:, :]
    hs = ((a + b) * np.float32(0.5), (a - b) * np.float32(0.5))  # beta form
    na, nb_ = nd[:, :H, :], nd[:, H:, :]

    maps = []
    for r in range(NCORES):
        dcols = np.empty((D, LB * S), dtype=BF)
        ncols = np.empty((D, LB * S), dtype=BF)
        for cl in range(LB):
            c = r * LB + cl
            if cl < K_BETA:
                left, right = hs[0][c], hs[1][c]  # (512, 128) each
            else:
                left, right = a[c], b[c]
            dcols[:, cl * S : cl * S + H] = left.T.astype(BF)
            dcols[:, cl * S + H : (cl + 1) * S] = right.T.astype(BF)
            # negs always kappa: raw halves
            ncols[:, cl * S : cl * S + H] = na[c].T.astype(BF)
            ncols[:, cl * S + H : (cl + 1) * S] = nb_[c].T.astype(BF)
        maps.append(
            {
                "qT": qtok,
                "qLocT": np.ascontiguousarray(qtok[:, r * LB * N : (r + 1) * LB * N]),
                "dT": dcols,
                "nT": ncols,
                "idenb": idenb,
                "ones4": ones4,
            }
        )
    return maps


def _epilogue(blocks, offset):
    """blocks: list of 8 (4, NCOLS) arrays -> final loss (float32 scalar)."""
    S_mat = np.empty((B, B), dtype=np.float64)
    negs = np.empty(B, dtype=np.float64)
    for r in range(NCORES):
        blk = np.asarray(blocks[r], dtype=np.float64)
        # blk[j, c*MT + m] = scores[4*m + j, r*LB + c]
        sc = blk[:, : MT * LB].reshape(4, LB, MT)  # (j, c, m)
        S_mat[:, r * LB : (r + 1) * LB] = np.transpose(sc, (2, 0, 1)).reshape(B, LB)
        # blk[j, MT*LB + g] = neg_score[local row 4g + j]
        for g in range(2):
            negs[r * LB + 4 * g : r * LB + 4 * g + 4] = blk[:, MT * LB + g]

    pos = np.diag(S_mat)
    x = (negs - pos) / TEMP
    loss1 = np.logaddexp(0.0, x).mean()  # stable softplus

    logits = S_mat / TEMP
    raw = np.arange(B) + int(offset)
    idx = np.where(raw < 0, raw + B, raw)
    valid = (idx >= 0) & (idx < B)
    row_max = logits.max(axis=1, keepdims=True)
    lse = np.log(np.exp(logits - row_max).sum(axis=1, keepdims=True)) + row_max
    logp = logits - lse
    picked = logp[np.arange(B), np.clip(idx, 0, B - 1)]
    picked = np.where(valid, picked, np.nan)
    ce = -picked.mean()

    return np.float32((loss1 + ce) / 2.0)


def kernel(query_embeddings, doc_embeddings, neg_doc_embeddings, offset):
    from concourse.bass_utils import run_bass_kernel_spmd

    q = np.asarray(query_embeddings, dtype=np.float32)
    d = np.asarray(doc_embeddings, dtype=np.float32)
    nd = np.asarray(neg_doc_embeddings, dtype=np.float32)
    assert q.shape == (B, N, D) and d.shape == (B, S, D) and nd.shape == (B, S, D)

    nc = get_nc()
    maps = _prep_inputs(q, d, nd)
    res = run_bass_kernel_spmd(nc, maps, core_ids=list(range(NCORES)))
    blocks = [res.results[r]["out"] for r in range(NCORES)]
    return _epilogue(blocks, offset)


def run_traced(query_embeddings, doc_embeddings, neg_doc_embeddings, offset, **trace_kw):
    """Like kernel() but returns (loss, BassKernelResults) for profiling."""
    from concourse.bass_utils import run_bass_kernel_spmd

    q = np.asarray(query_embeddings, dtype=np.float32)
    d = np.asarray(doc_embeddings, dtype=np.float32)
    nd = np.asarray(neg_doc_embeddings, dtype=np.float32)
    nc = get_nc()
    maps = _prep_inputs(q, d, nd)
    res = run_bass_kernel_spmd(
        nc, maps, core_ids=list(range(NCORES)), trace=True, **trace_kw
    )
    blocks = [res.results[r]["out"] for r in range(NCORES)]
    return _epilogue(blocks, offset), res


# revision 7
# speedup vs baseline: 1.3727x; 1.3727x over previous
"""ColBERT negative-CE loss on 8 Trainium2 NeuronCores (Bass/Tile).

Problem (hardcoded shapes): B=64, N=32 query tokens, S=1024 doc tokens, D=128.
  pos/neg paired MaxSim + in-batch (b x c) MaxSim cross-entropy, T=0.02.

Strategy (v2):
  * Shard the in-batch score matrix by DOC COLUMNS: core r computes
    scores[:, r*8:(r+1)*8] (all 64 query rows vs its 8 docs) plus the paired
    neg scores for its own 8 batch rows. pos_scores = diagonal of scores.
  * All matmuls in bf16 (inputs quantized host-side; ~1e-3 final rel err).
  * Each (m-tile, doc) tile = 1024 raw scores per lane in 2 PSUM banks.
    Two consumer pipelines, balanced across engines:
      kappa: ScalarE copies bank B -> SBUF; a CUSTOM DVE op (registered at
        runtime into concourse.dve_ops) computes max(bankA, copyB) and
        max-reduces it in ONE 2-port DVE pass (~0.73us/tile on DVE only).
      beta: host pre-computes half-doc sum/diff; PE computes P=q@hsum,
        Q=q@hdif; ScalarE abs(Q)->bf16 W; PE accumulates I@W onto P
        (max(a,b) = (a+b)/2 + |a-b|/2); DVE reduces two merged banks in one
        [128,2,512] pass (~0.63us/tile DVE, +0.25us PE).
    K_BETA docs per core take the beta path to offload DVE onto PE/ACT.
  * Neg tiles pack 4 batch rows (32 partitions each) into full 128-partition
    banks, handled by the kappa path (2 tile-units for 8 rows).
  * Final sum over the 32 query tokens per row: ones-block matmul ->
    [4, 130] result, DMA out; O(64x64) softmax/softplus epilogue on host.
"""

import numpy as np

B = 64
N = 32  # query tokens per row
S = 1024  # doc tokens
D = 128
NCORES = 8
LB = B // NCORES  # 8 docs (and batch rows) per core
H = S // 2  # 512, half-doc
MT = (B * N) // 128  # 16 m-tiles of 128 query tokens
TEMP = 0.02
K_BETA = 4  # docs 0..K_BETA-1 per core use the beta (sum/diff) path
NCOLS = MT * LB + 2  # 128 in-batch cols + 2 packed neg cols = 130

_NC_CACHE = {}


def _register_ttmax():
    """Register a custom DVE op: out = max(in0, in1); accum_out = row-max."""
    import concourse.dve_ops as dve_ops
    from concourse.dve_spec import AluOp, Spec, Src0, Src1, lower, maxx
    from concourse.dve_uop import DveOpSpec

    for op in dve_ops.OPS:
        if op.name == "TT_MAX_RED_ANT":
            return op

    def _ref(in0, in1, s0, s1, imm2):
        b = np.maximum(in0, in1).astype(np.float32)
        return b, b.reshape(b.shape[0], -1).max(axis=-1, keepdims=True)

    spec = Spec(body=maxx(Src0, Src1), accum=AluOp.MAX, reference=_ref)
    name = "TT_MAX_RED_ANT"
    row = dve_ops._CUSTOM_DVE_ROW_BASE + len(dve_ops.OPS)
    shas = {}
    for ver in ("v3", "v4"):
        uops = lower(spec, ver=ver)
        shas[ver] = DveOpSpec(name=name, opcode=row, uops=uops, rd1_en=True).sha(ver)
    op = dve_ops.DveOp(name, spec, subdim=False, uops_sha=shas)
    dve_ops.OPS.append(op)
    dve_ops._SUB_OPCODE_FOR_NAME[name] = row
    dve_ops.CUSTOM_DVE_SPECS[name] = spec
    return op


def _build_nc2():
    import concourse.bacc as bacc
    import concourse.mybir as mybir
    import concourse.tile as tile

    F32 = mybir.dt.float32
    F32R = mybir.dt.float32r
    BF16 = mybir.dt.bfloat16
    X = mybir.AxisListType.X
    ABS = mybir.ActivationFunctionType.Abs

    TTM = _register_ttmax()

    nc = bacc.Bacc("TRN2", target_bir_lowering=False, debug=False)

    qT = nc.dram_tensor("qT", [128, B * N], BF16, kind="ExternalInput").ap()
    qLocT = nc.dram_tensor("qLocT", [128, LB * N], BF16, kind="ExternalInput").ap()
    dT = nc.dram_tensor("dT", [128, LB * S], BF16, kind="ExternalInput").ap()
    nT = nc.dram_tensor("nT", [128, LB * S], BF16, kind="ExternalInput").ap()
    idenb = nc.dram_tensor("idenb", [128, 128], BF16, kind="ExternalInput").ap()
    ones4 = nc.dram_tensor("ones4", [128, 4], F32, kind="ExternalInput").ap()
    out = nc.dram_tensor("out", [4, NCOLS], F32, kind="ExternalOutput").ap()

    with tile.TileContext(nc) as tc:
        with (
            tc.tile_pool(name="consts", bufs=1) as consts,
            tc.tile_pool(name="wbuf", bufs=3) as wbuf_p,
            tc.tile_pool(name="vbuf", bufs=3) as vbuf_p,
            tc.tile_pool(name="scrp", bufs=2) as scr_p,
            tc.tile_pool(name="psumA", bufs=2, space="PSUM") as psA_p,
            tc.tile_pool(name="psumB", bufs=2, space="PSUM") as psB_p,
        ):
            q_t = consts.tile([128, B * N], BF16, tag="q")
            ql_t = consts.tile([128, LB * N], BF16, tag="ql")
            d_t = consts.tile([128, LB * S], BF16, tag="d")
            n_t = consts.tile([128, LB * S], BF16, tag="n")
            id_t = consts.tile([128, 128], BF16, tag="id")
            ones_t = consts.tile([128, 4], F32, tag="ones")
            mx = consts.tile([128, NCOLS], F32, tag="mx")
            nc.vector.memset(mx[:], 0.0)

            doc_order = []
            bdocs = list(range(K_BETA))
            kdocs = list(range(K_BETA, LB))
            while bdocs or kdocs:
                if bdocs:
                    doc_order.append(("b", bdocs.pop(0)))
                if kdocs:
                    doc_order.append(("k", kdocs.pop(0)))
                if kdocs:
                    doc_order.append(("k", kdocs.pop(0)))

            nc.sync.dma_start(q_t[:], qT[:])
            nc.sync.dma_start(id_t[:], idenb[:])
            nc.sync.dma_start(ones_t[:], ones4[:])
            for _, c in doc_order:
                nc.sync.dma_start(
                    d_t[:, c * S : (c + 1) * S], dT[:, c * S : (c + 1) * S]
                )
            nc.sync.dma_start(ql_t[:], qLocT[:])
            nc.sync.dma_start(n_t[:], nT[:])

            wz = consts.tile([128, 128], BF16, tag="wz")
            nc.vector.memset(wz[:].bitcast(mybir.dt.uint16), 0)
            wps = psA_p.tile([128, 1024], F32, tag="pa", name="warm")
            for _ in range(12):
                nc.tensor.matmul(
                    wps[:, 0:128], wz[:], wz[:], start=True, stop=True,
                    skip_group_check=True,
                )

            def q_ap(m):
                return q_t[:, m * 128 : (m + 1) * 128]

            # deferred beta merge+reduce closures (software pipeline: PE
            # never head-of-line blocks waiting on the ScalarE abs)
            pend = []

            def flush_pend():
                if pend:
                    pend.pop(0)()

            def beta_pair(c, m):
                ppP = psA_p.tile([128, 1024], F32, tag="pa")
                ppQ = psB_p.tile([128, 1024], F32, tag="pb")
                hs = d_t[:, c * S : c * S + H]
                hd = d_t[:, c * S + H : (c + 1) * S]
                nc.tensor.matmul(ppQ[:, 0:512], q_ap(m), hd, start=True, stop=True)
                nc.tensor.matmul(ppQ[:, 512:1024], q_ap(m + 1), hd, start=True, stop=True)
                nc.tensor.matmul(ppP[:, 0:512], q_ap(m), hs, start=True, stop=False)
                nc.tensor.matmul(ppP[:, 512:1024], q_ap(m + 1), hs, start=True, stop=False)
                w = wbuf_p.tile([128, 1024], BF16, tag="w")
                nc.scalar.activation(w[:], ppQ[:], ABS)
                col = c * MT + m

                def fin(ppP=ppP, w=w, col=col):
                    nc.tensor.matmul(ppP[:, 0:512], id_t[:], w[:, 0:512], start=False, stop=True)
                    nc.tensor.matmul(ppP[:, 512:1024], id_t[:], w[:, 512:1024], start=False, stop=True)
                    nc.vector.reduce_max(
                        mx[:, col : col + 2],
                        ppP[:].rearrange("p (w k) -> p w k", w=2),
                        axis=X,
                    )

                pend.append(fin)

            def kappa_pair(c, m):
                ppA = psA_p.tile([128, 1024], F32, tag="pa")
                ppB = psB_p.tile([128, 1024], F32, tag="pb")
                da = d_t[:, c * S : c * S + H]
                db = d_t[:, c * S + H : (c + 1) * S]
                nc.tensor.matmul(ppB[:, 0:512], q_ap(m), db, start=True, stop=True)
                nc.tensor.matmul(ppB[:, 512:1024], q_ap(m + 1), db, start=True, stop=True)
                nc.tensor.matmul(ppA[:, 0:512], q_ap(m), da, start=True, stop=True)
                nc.tensor.matmul(ppA[:, 512:1024], q_ap(m + 1), da, start=True, stop=True)
                v = vbuf_p.tile([128, 1024], F32, tag="v")
                nc.scalar.copy(v[:], ppB[:])
                col = c * MT + m
                scr = scr_p.tile([128, 512], F32, tag="scr")
                nc.vector._custom_dve(
                    TTM,
                    out=scr[:],
                    accum_out=mx[:, col : col + 1],
                    in0=ppA[:, 0:512],
                    in1=v[:, 0:512],
                )
                scr2 = scr_p.tile([128, 512], F32, tag="scr")
                nc.vector._custom_dve(
                    TTM,
                    out=scr2[:],
                    accum_out=mx[:, col + 1 : col + 2],
                    in0=ppA[:, 512:1024],
                    in1=v[:, 512:1024],
                )

            def neg_units():
                # 8 local rows; A-banks: [rows0-3 | rows4-7], B likewise
                ppA = psA_p.tile([128, 1024], F32, tag="pa")
                ppB = psB_p.tile([128, 1024], F32, tag="pb")
                for g in range(2):
                    for j in range(4):
                        b = 4 * g + j
                        lhs = ql_t[:, b * N : (b + 1) * N]
                        na = n_t[:, b * S : b * S + H]
                        nb = n_t[:, b * S + H : (b + 1) * S]
                        nc.tensor.matmul(
                            ppB[32 * j : 32 * j + 32, g * 512 : (g + 1) * 512],
                            lhs, nb, start=True, stop=True,
                            tile_position=(0, 32 * j),
                        )
                        nc.tensor.matmul(
                            ppA[32 * j : 32 * j + 32, g * 512 : (g + 1) * 512],
                            lhs, na, start=True, stop=True,
                            tile_position=(0, 32 * j),
                        )
                v = vbuf_p.tile([128, 1024], F32, tag="v")
                nc.scalar.copy(v[:], ppB[:])
                for g in range(2):
                    scr = scr_p.tile([128, 512], F32, tag="scr")
                    nc.vector._custom_dve(
                        TTM,
                        out=scr[:],
                        accum_out=mx[:, MT * LB + g : MT * LB + g + 1],
                        in0=ppA[:, g * 512 : (g + 1) * 512],
                        in1=v[:, g * 512 : (g + 1) * 512],
                    )

            for kind, c in doc_order:
                for m in range(0, MT, 2):
                    if kind == "b":
                        beta_pair(c, m)
                    else:
                        kappa_pair(c, m)
                    flush_pend()
            while pend:
                flush_pend()
            neg_units()

            # final: sum the 32 tokens of each row j via ones-block matmul
            psf = psA_p.tile([128, 1024], F32, tag="pa")
            nc.tensor.matmul(
                psf[0:4, 0:112], ones_t[:], mx[:, 0:112], start=True, stop=True,
            )
            nc.tensor.matmul(
                psf[0:4, 112:NCOLS], ones_t[:], mx[:, 112:NCOLS],
                start=True, stop=True,
            )
            out_sb = consts.tile([4, NCOLS], F32, tag="outsb")
            nc.scalar.copy(out_sb[:], psf[0:4, 0:NCOLS])
            nc.sync.dma_start(out[:], out_sb[:])

    nc.compile()
    return nc


def get_nc():
    if "nc" not in _NC_CACHE:
        _NC_CACHE["nc"] = _build_nc2()
    return _NC_CACHE["nc"]


def _prep_inputs(q, d, nd):
    """Build the 8 per-core input maps (bf16)."""
    import ml_dtypes

    BF = ml_dtypes.bfloat16
    qtok = np.ascontiguousarray(q.reshape(B * N, D).T).astype(BF)  # (128, 2048)
    idenb = np.eye(128, dtype=np.float32).astype(BF)
    ones4 = (np.arange(128)[:, None] // 32 == np.arange(4)[None, :]).astype(np.float32)

    a, b = d[:, :H, :], d[:, H:, :]
    hs = ((a + b) * np.float32(0.5), (a - b) * np.float32(0.5))  # beta form
    na, nb_ = nd[:, :H, :], nd[:, H:, :]

    maps = []
    for r in range(NCORES):
        dcols = np.empty((D, LB * S), dtype=BF)
        ncols = np.empty((D, LB * S), dtype=BF)
        for cl in range(LB):
            c = r * LB + cl
            if cl < K_BETA:
                left, right = hs[0][c], hs[1][c]  # (512, 128) each
            else:
                left, right = a[c], b[c]
            dcols[:, cl * S : cl * S + H] = left.T.astype(BF)
            dcols[:, cl * S + H : (cl + 1) * S] = right.T.astype(BF)
            # negs always kappa: raw halves
            ncols[:, cl * S : cl * S + H] = na[c].T.astype(BF)
            ncols[:, cl * S + H : (cl + 1) * S] = nb_[c].T.astype(BF)
        maps.append(
            {
                "qT": qtok,
                "qLocT": np.ascontiguousarray(qtok[:, r * LB * N : (r + 1) * LB * N]),
                "dT": dcols,
                "nT": ncols,
                "idenb": idenb,
                "ones4": ones4,
            }
        )
    return maps


def _epilogue(blocks, offset):
    """blocks: list of 8 (4, NCOLS) arrays -> final loss (float32 scalar)."""
    S_mat = np.empty((B, B), dtype=np.float64)
    negs = np.empty(B, dtype=np.float64)
    for r in range(NCORES):
        blk = np.asarray(blocks[r], dtype=np.float64)
        # blk[j, c*MT + m] = scores[4*m + j, r*LB + c]
        sc = blk[:, : MT * LB].reshape(4, LB, MT)  # (j, c, m)
        S_mat[:, r * LB : (r + 1) * LB] = np.transpose(sc, (2, 0, 1)).reshape(B, LB)
        # blk[j, MT*LB + g] = neg_score[local row 4g + j]
        for g in range(2):
            negs[r * LB + 4 * g : r * LB + 4 * g + 4] = blk[:, MT * LB + g]

    pos = np.diag(S_mat)
    x = (negs - pos) / TEMP
    loss1 = np.logaddexp(0.0, x).mean()  # stable softplus

    logits = S_mat / TEMP
    raw = np.arange(B) + int(offset)
    idx = np.where(raw < 0, raw + B, raw)
    valid = (idx >= 0) & (idx < B)
    row_max = logits.max(axis=1, keepdims=True)
    lse = np.log(np.exp(logits - row_max).sum(axis=1, keepdims=True)) + row_max
    logp = logits - lse
    picked = logp[np.arange(B), np.clip(idx, 0, B - 1)]
    picked = np.where(valid, picked, np.nan)
    ce = -picked.mean()

    return np.float32((loss1 + ce) / 2.0)


def kernel(query_embeddings, doc_embeddings, neg_doc_embeddings, offset):
    from concourse.bass_utils import run_bass_kernel_spmd

    q = np.asarray(query_embeddings, dtype=np.float32)
    d = np.asarray(doc_embeddings, dtype=np.float32)
    nd = np.asarray(neg_doc_embeddings, dtype=np.float32)
    assert q.shape == (B, N, D) and d.shape == (B, S, D) and nd.shape == (B, S, D)

    nc = get_nc()
    maps = _prep_inputs(q, d, nd)
    res = run_bass_kernel_spmd(nc, maps, core_ids=list(range(NCORES)))
    blocks = [res.results[r]["out"] for r in range(NCORES)]
    return _epilogue(blocks, offset)


def run_traced(query_embeddings, doc_embeddings, neg_doc_embeddings, offset, **trace_kw):
    """Like kernel() but returns (loss, BassKernelResults) for profiling."""
    from concourse.bass_utils import run_bass_kernel_spmd

    q = np.asarray(query_embeddings, dtype=np.float32)
    d = np.asarray(doc_embeddings, dtype=np.float32)
    nd = np.asarray(neg_doc_embeddings, dtype=np.float32)
    nc = get_nc()
    maps = _prep_inputs(q, d, nd)
    res = run_bass_kernel_spmd(
        nc, maps, core_ids=list(range(NCORES)), trace=True, **trace_kw
    )
    blocks = [res.results[r]["out"] for r in range(NCORES)]
    return _epilogue(blocks, offset), res


# revision 8
# speedup vs baseline: 1.3751x; 1.0018x over previous
"""ColBERT negative-CE loss on 8 Trainium2 NeuronCores (Bass/Tile).

Problem (hardcoded shapes): B=64, N=32 query tokens, S=1024 doc tokens, D=128.
  pos/neg paired MaxSim + in-batch (b x c) MaxSim cross-entropy, T=0.02.

Strategy (v2):
  * Shard the in-batch score matrix by DOC COLUMNS: core r computes
    scores[:, r*8:(r+1)*8] (all 64 query rows vs its 8 docs) plus the paired
    neg scores for its own 8 batch rows. pos_scores = diagonal of scores.
  * All matmuls in bf16 (inputs quantized host-side; ~1e-3 final rel err).
  * Each (m-tile, doc) tile = 1024 raw scores per lane in 2 PSUM banks.
    Two consumer pipelines, balanced across engines:
      kappa: ScalarE copies bank B -> SBUF; a CUSTOM DVE op (registered at
        runtime into concourse.dve_ops) computes max(bankA, copyB) and
        max-reduces it in ONE 2-port DVE pass (~0.73us/tile on DVE only).
      beta: host pre-computes half-doc sum/diff; PE computes P=q@hsum,
        Q=q@hdif; ScalarE abs(Q)->bf16 W; PE accumulates I@W onto P
        (max(a,b) = (a+b)/2 + |a-b|/2); DVE reduces two merged banks in one
        [128,2,512] pass (~0.63us/tile DVE, +0.25us PE).
    K_BETA docs per core take the beta path to offload DVE onto PE/ACT.
  * Neg tiles pack 4 batch rows (32 partitions each) into full 128-partition
    banks, handled by the kappa path (2 tile-units for 8 rows).
  * Final sum over the 32 query tokens per row: ones-block matmul ->
    [4, 130] result, DMA out; O(64x64) softmax/softplus epilogue on host.
"""

import numpy as np

B = 64
N = 32  # query tokens per row
S = 1024  # doc tokens
D = 128
NCORES = 8
LB = B // NCORES  # 8 docs (and batch rows) per core
H = S // 2  # 512, half-doc
MT = (B * N) // 128  # 16 m-tiles of 128 query tokens
TEMP = 0.02
K_BETA = 4  # docs 0..K_BETA-1 per core use the beta (sum/diff) path
NCOLS = MT * LB + 2  # 128 in-batch cols + 2 packed neg cols = 130

_NC_CACHE = {}


def _register_ttmax():
    """Register a custom DVE op: out = max(in0, in1); accum_out = row-max."""
    import concourse.dve_ops as dve_ops
    from concourse.dve_spec import AluOp, Spec, Src0, Src1, lower, maxx
    from concourse.dve_uop import DveOpSpec

    for op in dve_ops.OPS:
        if op.name == "TT_MAX_RED_ANT":
            return op

    def _ref(in0, in1, s0, s1, imm2):
        b = np.maximum(in0, in1).astype(np.float32)
        return b, b.reshape(b.shape[0], -1).max(axis=-1, keepdims=True)

    spec = Spec(body=maxx(Src0, Src1), accum=AluOp.MAX, reference=_ref)
    name = "TT_MAX_RED_ANT"
    row = dve_ops._CUSTOM_DVE_ROW_BASE + len(dve_ops.OPS)
    shas = {}
    for ver in ("v3", "v4"):
        uops = lower(spec, ver=ver)
        shas[ver] = DveOpSpec(name=name, opcode=row, uops=uops, rd1_en=True).sha(ver)
    op = dve_ops.DveOp(name, spec, subdim=False, uops_sha=shas)
    dve_ops.OPS.append(op)
    dve_ops._SUB_OPCODE_FOR_NAME[name] = row
    dve_ops.CUSTOM_DVE_SPECS[name] = spec
    return op


def _build_nc2():
    import concourse.bacc as bacc
    import concourse.mybir as mybir
    import concourse.tile as tile

    F32 = mybir.dt.float32
    F32R = mybir.dt.float32r
    BF16 = mybir.dt.bfloat16
    X = mybir.AxisListType.X
    ABS = mybir.ActivationFunctionType.Abs

    TTM = _register_ttmax()

    nc = bacc.Bacc("TRN2", target_bir_lowering=False, debug=False)

    qT = nc.dram_tensor("qT", [128, B * N], BF16, kind="ExternalInput").ap()
    qLocT = nc.dram_tensor("qLocT", [128, LB * N], BF16, kind="ExternalInput").ap()
    dT = nc.dram_tensor("dT", [128, LB * S], BF16, kind="ExternalInput").ap()
    nT = nc.dram_tensor("nT", [128, LB * S], BF16, kind="ExternalInput").ap()
    idenb = nc.dram_tensor("idenb", [128, 128], BF16, kind="ExternalInput").ap()
    ones4 = nc.dram_tensor("ones4", [128, 4], F32, kind="ExternalInput").ap()
    out = nc.dram_tensor("out", [4, NCOLS], F32, kind="ExternalOutput").ap()

    with tile.TileContext(nc) as tc:
        with (
            tc.tile_pool(name="consts", bufs=1) as consts,
            tc.tile_pool(name="wbuf", bufs=3) as wbuf_p,
            tc.tile_pool(name="vbuf", bufs=3) as vbuf_p,
            tc.tile_pool(name="scrp", bufs=2) as scr_p,
            tc.tile_pool(name="psumA", bufs=2, space="PSUM") as psA_p,
            tc.tile_pool(name="psumB", bufs=2, space="PSUM") as psB_p,
        ):
            q_t = consts.tile([128, B * N], BF16, tag="q")
            ql_t = consts.tile([128, LB * N], BF16, tag="ql")
            d_ts = [
                consts.tile([128, S], BF16, tag=f"d{c}", name=f"d{c}") for c in range(LB)
            ]
            n_t = consts.tile([128, LB * S], BF16, tag="n")
            id_t = consts.tile([128, 128], BF16, tag="id")
            ones_t = consts.tile([128, 4], F32, tag="ones")
            mx = consts.tile([128, NCOLS], F32, tag="mx")
            nc.vector.memset(mx[:], 0.0)

            doc_order = []
            bdocs = list(range(K_BETA))
            kdocs = list(range(K_BETA, LB))
            while bdocs or kdocs:
                if bdocs:
                    doc_order.append(("b", bdocs.pop(0)))
                if kdocs:
                    doc_order.append(("k", kdocs.pop(0)))
                if kdocs:
                    doc_order.append(("k", kdocs.pop(0)))

            nc.sync.dma_start(q_t[:], qT[:])
            nc.sync.dma_start(id_t[:], idenb[:])
            nc.sync.dma_start(ones_t[:], ones4[:])
            for _, c in doc_order:
                nc.sync.dma_start(d_ts[c][:], dT[:, c * S : (c + 1) * S])
            nc.sync.dma_start(ql_t[:], qLocT[:])
            nc.sync.dma_start(n_t[:], nT[:])

            wz = consts.tile([128, 128], BF16, tag="wz")
            nc.vector.memset(wz[:].bitcast(mybir.dt.uint16), 0)
            wps = psA_p.tile([128, 1024], F32, tag="pa", name="warm")
            for _ in range(12):
                nc.tensor.matmul(
                    wps[:, 0:128], wz[:], wz[:], start=True, stop=True,
                    skip_group_check=True,
                )

            def q_ap(m):
                return q_t[:, m * 128 : (m + 1) * 128]

            # deferred beta merge+reduce closures (software pipeline: PE
            # never head-of-line blocks waiting on the ScalarE abs)
            pend = []

            def flush_pend():
                if pend:
                    pend.pop(0)()

            def beta_pair(c, m):
                ppP = psA_p.tile([128, 1024], F32, tag="pa")
                ppQ = psB_p.tile([128, 1024], F32, tag="pb")
                hs = d_ts[c][:, 0:H]
                hd = d_ts[c][:, H:S]
                nc.tensor.matmul(ppQ[:, 0:512], q_ap(m), hd, start=True, stop=True)
                nc.tensor.matmul(ppQ[:, 512:1024], q_ap(m + 1), hd, start=True, stop=True)
                nc.tensor.matmul(ppP[:, 0:512], q_ap(m), hs, start=True, stop=False)
                nc.tensor.matmul(ppP[:, 512:1024], q_ap(m + 1), hs, start=True, stop=False)
                w = wbuf_p.tile([128, 1024], BF16, tag="w")
                nc.scalar.activation(w[:], ppQ[:], ABS)
                col = c * MT + m

                def fin(ppP=ppP, w=w, col=col):
                    nc.tensor.matmul(ppP[:, 0:512], id_t[:], w[:, 0:512], start=False, stop=True)
                    nc.tensor.matmul(ppP[:, 512:1024], id_t[:], w[:, 512:1024], start=False, stop=True)
                    nc.vector.reduce_max(
                        mx[:, col : col + 2],
                        ppP[:].rearrange("p (w k) -> p w k", w=2),
                        axis=X,
                    )

                pend.append(fin)

            def kappa_pair(c, m):
                ppA = psA_p.tile([128, 1024], F32, tag="pa")
                ppB = psB_p.tile([128, 1024], F32, tag="pb")
                da = d_ts[c][:, 0:H]
                db = d_ts[c][:, H:S]
                nc.tensor.matmul(ppB[:, 0:512], q_ap(m), db, start=True, stop=True)
                nc.tensor.matmul(ppB[:, 512:1024], q_ap(m + 1), db, start=True, stop=True)
                nc.tensor.matmul(ppA[:, 0:512], q_ap(m), da, start=True, stop=True)
                nc.tensor.matmul(ppA[:, 512:1024], q_ap(m + 1), da, start=True, stop=True)
                v = vbuf_p.tile([128, 1024], F32, tag="v")
                nc.scalar.copy(v[:], ppB[:])
                col = c * MT + m
                scr = scr_p.tile([128, 512], F32, tag="scr")
                nc.vector._custom_dve(
                    TTM,
                    out=scr[:],
                    accum_out=mx[:, col : col + 1],
                    in0=ppA[:, 0:512],
                    in1=v[:, 0:512],
                )
                scr2 = scr_p.tile([128, 512], F32, tag="scr")
                nc.vector._custom_dve(
                    TTM,
                    out=scr2[:],
                    accum_out=mx[:, col + 1 : col + 2],
                    in0=ppA[:, 512:1024],
                    in1=v[:, 512:1024],
                )

            def neg_units():
                # 8 local rows; A-banks: [rows0-3 | rows4-7], B likewise
                ppA = psA_p.tile([128, 1024], F32, tag="pa")
                ppB = psB_p.tile([128, 1024], F32, tag="pb")
                for g in range(2):
                    for j in range(4):
                        b = 4 * g + j
                        lhs = ql_t[:, b * N : (b + 1) * N]
                        na = n_t[:, b * S : b * S + H]
                        nb = n_t[:, b * S + H : (b + 1) * S]
                        nc.tensor.matmul(
                            ppB[32 * j : 32 * j + 32, g * 512 : (g + 1) * 512],
                            lhs, nb, start=True, stop=True,
                            tile_position=(0, 32 * j),
                        )
                        nc.tensor.matmul(
                            ppA[32 * j : 32 * j + 32, g * 512 : (g + 1) * 512],
                            lhs, na, start=True, stop=True,
                            tile_position=(0, 32 * j),
                        )
                v = vbuf_p.tile([128, 1024], F32, tag="v")
                nc.scalar.copy(v[:], ppB[:])
                for g in range(2):
                    scr = scr_p.tile([128, 512], F32, tag="scr")
                    nc.vector._custom_dve(
                        TTM,
                        out=scr[:],
                        accum_out=mx[:, MT * LB + g : MT * LB + g + 1],
                        in0=ppA[:, g * 512 : (g + 1) * 512],
                        in1=v[:, g * 512 : (g + 1) * 512],
                    )

            for kind, c in doc_order:
                for m in range(0, MT, 2):
                    if kind == "b":
                        beta_pair(c, m)
                    else:
                        kappa_pair(c, m)
                    flush_pend()
            while pend:
                flush_pend()
            neg_units()

            # final: sum the 32 tokens of each row j via ones-block matmul
            psf = psA_p.tile([128, 1024], F32, tag="pa")
            nc.tensor.matmul(
                psf[0:4, 0:112], ones_t[:], mx[:, 0:112], start=True, stop=True,
            )
            nc.tensor.matmul(
                psf[0:4, 112:NCOLS], ones_t[:], mx[:, 112:NCOLS],
                start=True, stop=True,
            )
            out_sb = consts.tile([4, NCOLS], F32, tag="outsb")
            nc.scalar.copy(out_sb[:], psf[0:4, 0:NCOLS])
            nc.sync.dma_start(out[:], out_sb[:])

    nc.compile()
    return nc


def get_nc():
    if "nc" not in _NC_CACHE:
        _NC_CACHE["nc"] = _build_nc2()
    return _NC_CACHE["nc"]


def _prep_inputs(q, d, nd):
    """Build the 8 per-core input maps (bf16)."""
    import ml_dtypes

    BF = ml_dtypes.bfloat16
    qtok = np.ascontiguousarray(q.reshape(B * N, D).T).astype(BF)  # (128, 2048)
    idenb = np.eye(128, dtype=np.float32).astype(BF)
    ones4 = (np.arange(128)[:, None] // 32 == np.arange(4)[None, :]).astype(np.float32)

    a, b = d[:, :H, :], d[:, H:, :]
    hs = ((a + b) * np.float32(0.5), (a - b) * np.float32(0.5))  # beta form
    na, nb_ = nd[:, :H, :], nd[:, H:, :]

    maps = []
    for r in range(NCORES):
        dcols = np.empty((D, LB * S), dtype=BF)
        ncols = np.empty((D, LB * S), dtype=BF)
        for cl in range(LB):
            c = r * LB + cl
            if cl < K_BETA:
                left, right = hs[0][c], hs[1][c]  # (512, 128) each
            else:
                left, right = a[c], b[c]
            dcols[:, cl * S : cl * S + H] = left.T.astype(BF)
            dcols[:, cl * S + H : (cl + 1) * S] = right.T.astype(BF)
            # negs always kappa: raw halves
            ncols[:, cl * S : cl * S + H] = na[c].T.astype(BF)
            ncols[:, cl * S + H : (cl + 1) * S] = nb_[c].T.astype(BF)
        maps.append(
            {
                "qT": qtok,
                "qLocT": np.ascontiguousarray(qtok[:, r * LB * N : (r + 1) * LB * N]),
                "dT": dcols,
                "nT": ncols,
                "idenb": idenb,
                "ones4": ones4,
            }
        )
    return maps


def _epilogue(blocks, offset):
    """blocks: list of 8 (4, NCOLS) arrays -> final loss (float32 scalar)."""
    S_mat = np.empty((B, B), dtype=np.float64)
    negs = np.empty(B, dtype=np.float64)
    for r in range(NCORES):
        blk = np.asarray(blocks[r], dtype=np.float64)
        # blk[j, c*MT + m] = scores[4*m + j, r*LB + c]
        sc = blk[:, : MT * LB].reshape(4, LB, MT)  # (j, c, m)
        S_mat[:, r * LB : (r + 1) * LB] = np.transpose(sc, (2, 0, 1)).reshape(B, LB)
        # blk[j, MT*LB + g] = neg_score[local row 4g + j]
        for g in range(2):
            negs[r * LB + 4 * g : r * LB + 4 * g + 4] = blk[:, MT * LB + g]

    pos = np.diag(S_mat)
    x = (negs - pos) / TEMP
    loss1 = np.logaddexp(0.0, x).mean()  # stable softplus

    logits = S_mat / TEMP
    raw = np.arange(B) + int(offset)
    idx = np.where(raw < 0, raw + B, raw)
    valid = (idx >= 0) & (idx < B)
    row_max = logits.max(axis=1, keepdims=True)
    lse = np.log(np.exp(logits - row_max).sum(axis=1, keepdims=True)) + row_max
    logp = logits - lse
    picked = logp[np.arange(B), np.clip(idx, 0, B - 1)]
    picked = np.where(valid, picked, np.nan)
    ce = -picked.mean()

    return np.float32((loss1 + ce) / 2.0)


def kernel(query_embeddings, doc_embeddings, neg_doc_embeddings, offset):
    from concourse.bass_utils import run_bass_kernel_spmd

    q = np.asarray(query_embeddings, dtype=np.float32)
    d = np.asarray(doc_embeddings, dtype=np.float32)
    nd = np.asarray(neg_doc_embeddings, dtype=np.float32)
    assert q.shape == (B, N, D) and d.shape == (B, S, D) and nd.shape == (B, S, D)

    nc = get_nc()
    maps = _prep_inputs(q, d, nd)
    res = run_bass_kernel_spmd(nc, maps, core_ids=list(range(NCORES)))
    blocks = [res.results[r]["out"] for r in range(NCORES)]
    return _epilogue(blocks, offset)


def run_traced(query_embeddings, doc_embeddings, neg_doc_embeddings, offset, **trace_kw):
    """Like kernel() but returns (loss, BassKernelResults) for profiling."""
    from concourse.bass_utils import run_bass_kernel_spmd

    q = np.asarray(query_embeddings, dtype=np.float32)
    d = np.asarray(doc_embeddings, dtype=np.float32)
    nd = np.asarray(neg_doc_embeddings, dtype=np.float32)
    nc = get_nc()
    maps = _prep_inputs(q, d, nd)
    res = run_bass_kernel_spmd(
        nc, maps, core_ids=list(range(NCORES)), trace=True, **trace_kw
    )
    blocks = [res.results[r]["out"] for r in range(NCORES)]
    return _epilogue(blocks, offset), res
